# revision 20
# baseline (speedup 1.0000x reference)
"""ComplEx scoring kernel for 8 Trainium2 NeuronCores.

Math: score[b, e] = Re(<h_b * r_b, conj(ent_e)>) with h = ent_emb[triples[:,0]],
r = rel_emb[triples[:,1]].  Writing ans_b = concat(re_h*re_r - im_h*im_r,
re_h*im_r + im_h*re_r) (shape [B, 512]), the score is exactly
score = ans @ ent_emb.T  — one [1024, 512] x [512, 200000] GEMM.

Strategy (vocab/tensor parallel along the entity axis):
  - host: tiny gather + complex multiply -> ans  (microseconds)
  - shard ent_emb rows 8 ways (25000/core, exact, no padding),
    pre-transposed + bf16-cast on host so the device streams contiguous
    [K=512, E] tiles
  - each core: score_shard[1024, 25000] = ansT.T @ entT via PE-array
    matmuls (bf16 in, fp32 PSUM accumulate); this is PE-bound at the
    bf16 roofline (~333 us), so the kernel minimizes PE idle time:
      * startup: the first psum group's operands (ent t0 + ansT m0)
        are the first DMAs issued, so the PE starts ~8 us in instead
        of ~19 us
      * group 0 runs t-outer / m-inner, so its DMA demand (~76 GB/s)
        stays below delivery rate and the PE never starves while the
        pipeline fills
      * steady state runs k-outer (weights switch once per 7 matmuls)
        with groups prefetched 2 deep
      * output stores are issued from the copyback engines (DVE/Act),
        keeping the SP queue free for input loads
      * the last block drains with per-tile, partition-split stores
  - host: concatenate the 8 column slabs
"""

import numpy as np
import ml_dtypes

NCORES = 8
NUM_ENT = 200000
EMB = 512
B = 1024
SHARD = NUM_ENT // NCORES      # 25000 entities per core (exact)
NTILE = 512
NGROUPS = 7
TILE_W = [NTILE] * 48 + [424]  # 49 tiles, sum = 25000
GROUP_TILES = [TILE_W[g * 7:(g + 1) * 7] for g in range(NGROUPS)]
GROUP_W = [sum(ws) for ws in GROUP_TILES]          # 6x3584 + 3496
GROUP_OFF = [NTILE * 7 * g for g in range(NGROUPS)]
GMAX = max(GROUP_W)            # 3584
KCH = EMB // 128               # 4 contraction chunks
MCH = B // 128                 # 8 batch chunks

_NC = None

# score values are ~1e-5 — subnormal in fp16.  Pre-scaling ans by 2**16 on
# the host puts the device-side scores in fp16's normal range, so the output
# can be stored/DMA'd as fp16 (half the write traffic); the host unscales.
OUT_SCALE = 2.0 ** 16


def _build_nc():
    import concourse.bacc as bacc
    import concourse.bass as bass
    import concourse.tile as tile
    from concourse import mybir

    ts, ds = bass.ts, bass.ds
    bf16 = mybir.dt.bfloat16
    f16 = mybir.dt.float16
    f32 = mybir.dt.float32

    nc = bacc.Bacc("TRN2", target_bir_lowering=False, debug=False)
    ansT = nc.dram_tensor("ansT", [EMB, B], bf16, kind="ExternalInput")
    entT = nc.dram_tensor("entT", [EMB, SHARD], bf16, kind="ExternalInput")
    score = nc.dram_tensor("score", [B, SHARD], f16, kind="ExternalOutput")

    with tile.TileContext(nc) as tc:
        with tc.tile_pool(name="const", bufs=1) as const_pool, \
             tc.tile_pool(name="entp", bufs=3 * KCH) as ent_pool, \
             tc.tile_pool(name="outp", bufs=10) as out_pool, \
             tc.tile_pool(name="ps", bufs=8, space="PSUM") as psum_pool:

            def load_group(g):
                # one tile (one DMA) per k-chunk; packets of a single DMA
                # spread across all 16 DMA engines, so big DMAs are fine
                w = GROUP_W[g]
                tiles = []
                for k in range(KCH):
                    t = ent_pool.tile([128, GMAX], bf16, name="ent_sb")
                    nc.sync.dma_start(t[:, :w],
                                      entT[ts(k, 128), ds(GROUP_OFF[g], w)])
                    tiles.append(t)
                return tiles

            # --- startup: first psum group's operands land first ---
            ansT_sb = const_pool.tile([128, KCH, B], bf16, name="ansT_sb")
            ent_g0 = [ent_pool.tile([128, GMAX], bf16, name="ent_sb")
                      for _ in range(KCH)]
            # critical path to the first matmul is SP's 620ns-per-DMA issue
            # rate: put ent t0 on SP and ansT on Act so they issue in parallel
            for k in range(KCH):                       # ent t0: 4 x 128KB
                nc.sync.dma_start(ent_g0[k][:, :NTILE],
                                  entT[ts(k, 128), ds(0, NTILE)])
            for k in range(KCH):                       # ansT m0: 4 x 32KB
                nc.scalar.dma_start(ansT_sb[:, k, :128], ansT[ts(k, 128), :128])
            for k in range(KCH):                       # ansT m1-3
                nc.scalar.dma_start(ansT_sb[:, k, 128:512],
                                    ansT[ts(k, 128), 128:512])
            for k in range(KCH):                       # ansT m4-7
                nc.scalar.dma_start(ansT_sb[:, k, 512:],
                                    ansT[ts(k, 128), 512:])
            # ent t1-6 per tile so tile t becomes usable after 512KB, not
            # after the whole 3MB remainder
            for t in range(1, 7):
                for k in range(KCH):
                    nc.sync.dma_start(ent_g0[k][:, ts(t, NTILE)],
                                      entT[ts(k, 128), ds(t * NTILE, NTILE)])
            ent_tiles = {0: ent_g0, 1: load_group(1), 2: load_group(2)}

            # warm up the PE while it waits for the first DMAs.  The HAM
            # clock governor upgrades 4/8 -> 8/8 only after ~5.5us of DENSE
            # full-width activity (skinny matmuls don't register), and the
            # dummies must hand off to real work with no idle gap, so their
            # count is tuned to end right at data-ready (~12us).  gpsimd
            # boots first (~2us vs ~7us for DVE), so it does the memset.
            warm_sb = const_pool.tile([128, NTILE], bf16, name="warm_sb")
            nc.gpsimd.memset(warm_sb[:], 0)
            ps_warm = psum_pool.tile([128, NTILE], f32, name="pst")
            for _ in range(10):
                nc.tensor.matmul(ps_warm[:], warm_sb[:, :128],
                                 warm_sb[:], start=True, stop=True)

            def copyback(eng, dst, src):
                if eng is nc.scalar:
                    eng.copy(dst, src)
                else:
                    eng.tensor_copy(out=dst, in_=src)

            def copyback2(dst, src, w):
                # split across both engines: psum frees ~340ns after the
                # tile's last matmul instead of up to ~2.7us (engine queue)
                h = (w // 2 + 63) // 64 * 64
                nc.vector.tensor_copy(out=dst[:, :h], in_=src[:, :h])
                nc.scalar.copy(dst[:, ds(h, w - h)], src[:, ds(h, w - h)])

            # --- group 0: t-outer / m-inner so DMA keeps up while the
            # pipeline fills (demand ~0.5MB per 6.8us of PE work) ---
            out_g0 = [out_pool.tile([128, GMAX], f16, name="out_sb")
                      for _ in range(MCH)]
            for t in range(7):
                for m in range(MCH):
                    ps = psum_pool.tile([128, NTILE], f32, name="pst")
                    for k in range(KCH):
                        nc.tensor.matmul(
                            ps[:],
                            ansT_sb[:, k, ts(m, 128)],
                            ent_g0[k][:, ts(t, NTILE)],
                            start=(k == 0),
                            stop=(k == KCH - 1),
                        )
                    copyback2(out_g0[m][:, ts(t, NTILE)], ps, NTILE)
            for m in range(MCH):
                nc.sync.dma_start(score[ts(m, 128), ds(0, GROUP_W[0])],
                                  out_g0[m][:, :GROUP_W[0]])

            # --- groups 1..6: k-outer steady state ---
            for g in range(1, NGROUPS):
                if g + 2 < NGROUPS:
                    ent_tiles[g + 2] = load_group(g + 2)
                ent_sb = ent_tiles.pop(g)
                widths = GROUP_TILES[g]
                offs = [sum(widths[:i]) for i in range(7)]
                goff = GROUP_OFF[g]
                gw = GROUP_W[g]
                for m in range(MCH):
                    last_block = (g == NGROUPS - 1) and (m == MCH - 1)
                    pss = [psum_pool.tile([128, NTILE], f32, name="pst")
                           for _ in range(7)]
                    out_sb = out_pool.tile([128, GMAX], f16, name="out_sb")
                    if not last_block:
                        # k outer: stationary weights switch once per 7
                        # matmuls instead of every matmul
                        for k in range(KCH):
                            lhsT = ansT_sb[:, k, ts(m, 128)]
                            for t in range(7):
                                nc.tensor.matmul(
                                    pss[t][:, :widths[t]],
                                    lhsT,
                                    ent_sb[k][:, ds(offs[t], widths[t])],
                                    start=(k == 0),
                                    stop=(k == KCH - 1),
                                )
                        for t in range(7):
                            copyback2(out_sb[:, ds(offs[t], widths[t])],
                                      pss[t], widths[t])
                        # stores from SP, which is idle mid-kernel: keeps the
                        # copy engines free and the store issue off their path
                        h0 = offs[4]
                        nc.sync.dma_start(score[ts(m, 128), ds(goff, h0)],
                                          out_sb[:, :h0])
                        nc.sync.dma_start(
                            score[ts(m, 128), ds(goff + h0, gw - h0)],
                            out_sb[:, ds(h0, gw - h0)])
                    else:
                        # final block: t-outer; progressive partition-split
                        # stores with >=2KB rows, issued from the (idle) SP
                        # queue so the drain never waits on the copy engines
                        store_after = {1: (0, 1024), 3: (1024, 1024),
                                       5: (2048, 1024), 6: (3072, gw - 3072)}
                        for t in range(7):
                            for k in range(KCH):
                                nc.tensor.matmul(
                                    pss[t][:, :widths[t]],
                                    ansT_sb[:, k, ts(m, 128)],
                                    ent_sb[k][:, ds(offs[t], widths[t])],
                                    start=(k == 0),
                                    stop=(k == KCH - 1),
                                )
                            copyback2(out_sb[:, ds(offs[t], widths[t])],
                                      pss[t], widths[t])
                            if t in store_after:
                                o, w = store_after[t]
                                for h in range(2):
                                    nc.sync.dma_start(
                                        score[ds(m * 128 + h * 64, 64),
                                              ds(goff + o, w)],
                                        out_sb[ds(h * 64, 64), ds(o, w)])
    nc.compile()
    return nc


def _get_nc():
    global _NC
    if _NC is None:
        _NC = _build_nc()
    return _NC


def _pmap(fn, n):
    from concurrent.futures import ThreadPoolExecutor
    with ThreadPoolExecutor(max_workers=n) as ex:
        list(ex.map(fn, range(n)))


def prepare_in_maps(triples, ent_emb, rel_emb):
    triples = np.asarray(triples)
    ent_emb = np.asarray(ent_emb, dtype=np.float32)
    rel_emb = np.asarray(rel_emb, dtype=np.float32)

    d = EMB // 2
    h = ent_emb[triples[:, 0].astype(np.int64)]
    r = rel_emb[triples[:, 1].astype(np.int64)]
    re_h, im_h = h[:, :d], h[:, d:]
    re_r, im_r = r[:, :d], r[:, d:]
    ans = np.empty((B, EMB), np.float32)
    ans[:, :d] = re_h * re_r - im_h * im_r
    ans[:, d:] = re_h * im_r + im_h * re_r
    ans *= np.float32(OUT_SCALE)
    ansT_bf = np.ascontiguousarray(ans.T).astype(ml_dtypes.bfloat16)

    ent_bf = np.empty(ent_emb.shape, dtype=ml_dtypes.bfloat16)
    shards = np.empty((NCORES, EMB, SHARD), dtype=ml_dtypes.bfloat16)

    def _cast(c):
        s = slice(c * SHARD, (c + 1) * SHARD)
        ent_bf[s] = ent_emb[s]

    def _shard(c):
        shards[c] = ent_bf[c * SHARD:(c + 1) * SHARD].T

    _pmap(_cast, NCORES)
    _pmap(_shard, NCORES)
    return [{"ansT": ansT_bf, "entT": shards[c]} for c in range(NCORES)]


def run_raw(in_maps, trace=False):
    from concourse import bass_utils
    return bass_utils.run_bass_kernel_spmd(
        _get_nc(), in_maps, core_ids=list(range(NCORES)), trace=trace
    )


def assemble(results):
    out = np.empty((B, NUM_ENT), np.float32)
    inv = np.float32(1.0 / OUT_SCALE)

    def _one(c):
        sh = results[c]["score"].astype(np.float32)
        sh *= inv
        out[:, c * SHARD:(c + 1) * SHARD] = sh
    _pmap(_one, NCORES)
    return out


def kernel(triples, ent_emb, rel_emb):
    in_maps = prepare_in_maps(triples, ent_emb, rel_emb)
    res = run_raw(in_maps)
    return assemble(res.results)


# revision 24
# speedup vs baseline: 1.0223x; 1.0223x over previous
"""ComplEx scoring kernel for 8 Trainium2 NeuronCores.

Math: score[b, e] = Re(<h_b * r_b, conj(ent_e)>) with h = ent_emb[triples[:,0]],
r = rel_emb[triples[:,1]].  Writing ans_b = concat(re_h*re_r - im_h*im_r,
re_h*im_r + im_h*re_r) (shape [B, 512]), the score is exactly
score = ans @ ent_emb.T  — one [1024, 512] x [512, 200000] GEMM.

Strategy (vocab/tensor parallel along the entity axis):
  - host: tiny gather + complex multiply -> ans  (microseconds)
  - shard ent_emb rows 8 ways (25000/core, exact, no padding),
    pre-transposed + bf16-cast on host so the device streams contiguous
    [K=512, E] tiles
  - each core: score_shard[1024, 25000] = ansT.T @ entT via PE-array
    matmuls (bf16 in, fp32 PSUM accumulate); this is PE-bound at the
    bf16 roofline (~333 us), so the kernel minimizes PE idle time:
      * startup: the first psum group's operands (ent t0 + ansT m0)
        are the first DMAs issued, so the PE starts ~8 us in instead
        of ~19 us
      * group 0 runs t-outer / m-inner, so its DMA demand (~76 GB/s)
        stays below delivery rate and the PE never starves while the
        pipeline fills
      * steady state runs k-outer (weights switch once per 7 matmuls)
        with groups prefetched 2 deep
      * output stores are issued from the copyback engines (DVE/Act),
        keeping the SP queue free for input loads
      * the last block drains with per-tile, partition-split stores
  - host: concatenate the 8 column slabs
"""

import numpy as np
import ml_dtypes

NCORES = 8
NUM_ENT = 200000
EMB = 512
B = 1024
SHARD = NUM_ENT // NCORES      # 25000 entities per core (exact)
NTILE = 512
NGROUPS = 7
TILE_W = [NTILE] * 48 + [424]  # 49 tiles, sum = 25000
GROUP_TILES = [TILE_W[g * 7:(g + 1) * 7] for g in range(NGROUPS)]
GROUP_W = [sum(ws) for ws in GROUP_TILES]          # 6x3584 + 3496
GROUP_OFF = [NTILE * 7 * g for g in range(NGROUPS)]
GMAX = max(GROUP_W)            # 3584
KCH = EMB // 128               # 4 contraction chunks
MCH = B // 128                 # 8 batch chunks

_NC = None

# score values are ~1e-5 — subnormal in fp16.  Pre-scaling ans by 2**16 on
# the host puts the device-side scores in fp16's normal range, so the output
# can be stored/DMA'd as fp16 (half the write traffic); the host unscales.
OUT_SCALE = 2.0 ** 16


def _build_nc():
    import concourse.bacc as bacc
    import concourse.bass as bass
    import concourse.tile as tile
    from concourse import mybir

    ts, ds = bass.ts, bass.ds
    bf16 = mybir.dt.bfloat16
    f16 = mybir.dt.float16
    f32 = mybir.dt.float32

    nc = bacc.Bacc("TRN2", target_bir_lowering=False, debug=False)
    ansT = nc.dram_tensor("ansT", [EMB, B], bf16, kind="ExternalInput")
    entT = nc.dram_tensor("entT", [EMB, SHARD], bf16, kind="ExternalInput")
    score = nc.dram_tensor("score", [B, SHARD], f16, kind="ExternalOutput")

    with tile.TileContext(nc) as tc:
        with tc.tile_pool(name="const", bufs=1) as const_pool, \
             tc.tile_pool(name="entp", bufs=3 * KCH) as ent_pool, \
             tc.tile_pool(name="outp", bufs=10) as out_pool, \
             tc.tile_pool(name="ps", bufs=8, space="PSUM") as psum_pool:

            def load_group(g):
                # one tile (one DMA) per k-chunk; packets of a single DMA
                # spread across all 16 DMA engines, so big DMAs are fine
                w = GROUP_W[g]
                tiles = []
                for k in range(KCH):
                    t = ent_pool.tile([128, GMAX], bf16, name="ent_sb")
                    nc.sync.dma_start(t[:, :w],
                                      entT[ts(k, 128), ds(GROUP_OFF[g], w)])
                    tiles.append(t)
                return tiles

            # --- startup: first psum group's operands land first ---
            ansT_sb = const_pool.tile([128, KCH, B], bf16, name="ansT_sb")
            ent_g0 = [ent_pool.tile([128, GMAX], bf16, name="ent_sb")
                      for _ in range(KCH)]
            # critical path to the first matmul is SP's 620ns-per-DMA issue
            # rate: put ent t0 on SP and ansT on Act so they issue in parallel
            for k in range(KCH):                       # ent t0: 4 x 128KB
                nc.sync.dma_start(ent_g0[k][:, :NTILE],
                                  entT[ts(k, 128), ds(0, NTILE)])
            for k in range(KCH):                       # ansT m0: 4 x 32KB
                nc.scalar.dma_start(ansT_sb[:, k, :128], ansT[ts(k, 128), :128])
            for k in range(KCH):                       # ansT m1-3
                nc.scalar.dma_start(ansT_sb[:, k, 128:512],
                                    ansT[ts(k, 128), 128:512])
            for k in range(KCH):                       # ansT m4-7
                nc.scalar.dma_start(ansT_sb[:, k, 512:],
                                    ansT[ts(k, 128), 512:])
            # ent t1-6 per tile so tile t becomes usable after 512KB, not
            # after the whole 3MB remainder
            for t in range(1, 7):
                for k in range(KCH):
                    nc.sync.dma_start(ent_g0[k][:, ts(t, NTILE)],
                                      entT[ts(k, 128), ds(t * NTILE, NTILE)])
            ent_tiles = {0: ent_g0, 1: load_group(1), 2: load_group(2)}

            # warm up the PE while it waits for the first DMAs.  The HAM
            # clock governor upgrades 4/8 -> 8/8 only after ~5.5us of DENSE
            # full-width activity (skinny matmuls don't register), and the
            # dummies must hand off to real work with no idle gap, so their
            # count is tuned to end right at data-ready (~12us).  gpsimd
            # boots first (~2us vs ~7us for DVE), so it does the memset.
            warm_sb = const_pool.tile([128, NTILE], bf16, name="warm_sb")
            nc.gpsimd.memset(warm_sb[:], 0)
            ps_warm = psum_pool.tile([128, NTILE], f32, name="pst")
            for _ in range(10):
                nc.tensor.matmul(ps_warm[:], warm_sb[:, :128],
                                 warm_sb[:], start=True, stop=True)

            def copyback(eng, dst, src):
                if eng is nc.scalar:
                    eng.copy(dst, src)
                else:
                    eng.tensor_copy(out=dst, in_=src)

            # --- group 0: t-outer / m-inner so DMA keeps up while the
            # pipeline fills (demand ~0.5MB per 6.8us of PE work) ---
            out_g0 = [out_pool.tile([128, GMAX], f16, name="out_sb")
                      for _ in range(MCH)]
            for t in range(7):
                for m in range(MCH):
                    ps = psum_pool.tile([128, NTILE], f32, name="pst")
                    for k in range(KCH):
                        nc.tensor.matmul(
                            ps[:],
                            ansT_sb[:, k, ts(m, 128)],
                            ent_g0[k][:, ts(t, NTILE)],
                            start=(k == 0),
                            stop=(k == KCH - 1),
                        )
                    eng = nc.vector if m % 2 == 0 else nc.scalar
                    copyback(eng, out_g0[m][:, ts(t, NTILE)], ps[:])
            for m in range(MCH):
                nc.sync.dma_start(score[ts(m, 128), ds(0, GROUP_W[0])],
                                  out_g0[m][:, :GROUP_W[0]])

            # --- groups 1..6: k-outer steady state ---
            for g in range(1, NGROUPS):
                if g + 2 < NGROUPS:
                    ent_tiles[g + 2] = load_group(g + 2)
                ent_sb = ent_tiles.pop(g)
                widths = GROUP_TILES[g]
                offs = [sum(widths[:i]) for i in range(7)]
                goff = GROUP_OFF[g]
                gw = GROUP_W[g]
                for m in range(MCH):
                    last_block = (g == NGROUPS - 1) and (m == MCH - 1)
                    pss = [psum_pool.tile([128, NTILE], f32, name="pst")
                           for _ in range(7)]
                    out_sb = out_pool.tile([128, GMAX], f16, name="out_sb")
                    if not last_block:
                        # k outer within each phase (weights switch once per
                        # phase-width), split into two phases so t0-t2 finish
                        # ~3us before block end: their copies start early and
                        # the psum bufs recycle with slack
                        for lo, hi in ((0, 3), (3, 7)):
                            for k in range(KCH):
                                lhsT = ansT_sb[:, k, ts(m, 128)]
                                for t in range(lo, hi):
                                    nc.tensor.matmul(
                                        pss[t][:, :widths[t]],
                                        lhsT,
                                        ent_sb[k][:, ds(offs[t], widths[t])],
                                        start=(k == 0),
                                        stop=(k == KCH - 1),
                                    )
                            for t in range(lo, hi):
                                eng = nc.vector if t % 2 == 0 else nc.scalar
                                copyback(eng,
                                         out_sb[:, ds(offs[t], widths[t])],
                                         pss[t][:, :widths[t]])
                        # stores from SP, which is idle mid-kernel: keeps the
                        # copy engines free and the store issue off their path
                        h0 = offs[4]
                        nc.sync.dma_start(score[ts(m, 128), ds(goff, h0)],
                                          out_sb[:, :h0])
                        nc.sync.dma_start(
                            score[ts(m, 128), ds(goff + h0, gw - h0)],
                            out_sb[:, ds(h0, gw - h0)])
                    else:
                        # final block: t-outer; progressive partition-split
                        # stores with >=2KB rows, issued from the (idle) SP
                        # queue so the drain never waits on the copy engines
                        store_after = {1: (0, 1024), 3: (1024, 1024),
                                       5: (2048, 1024), 6: (3072, gw - 3072)}
                        for t in range(7):
                            for k in range(KCH):
                                nc.tensor.matmul(
                                    pss[t][:, :widths[t]],
                                    ansT_sb[:, k, ts(m, 128)],
                                    ent_sb[k][:, ds(offs[t], widths[t])],
                                    start=(k == 0),
                                    stop=(k == KCH - 1),
                                )
                            eng = nc.vector if t % 2 == 0 else nc.scalar
                            copyback(eng, out_sb[:, ds(offs[t], widths[t])],
                                     pss[t][:, :widths[t]])
                            if t in store_after:
                                o, w = store_after[t]
                                for h in range(2):
                                    nc.sync.dma_start(
                                        score[ds(m * 128 + h * 64, 64),
                                              ds(goff + o, w)],
                                        out_sb[ds(h * 64, 64), ds(o, w)])
    nc.compile()
    return nc


def _get_nc():
    global _NC
    if _NC is None:
        _NC = _build_nc()
    return _NC


def _pmap(fn, n):
    from concurrent.futures import ThreadPoolExecutor
    with ThreadPoolExecutor(max_workers=n) as ex:
        list(ex.map(fn, range(n)))


def prepare_in_maps(triples, ent_emb, rel_emb):
    triples = np.asarray(triples)
    ent_emb = np.asarray(ent_emb, dtype=np.float32)
    rel_emb = np.asarray(rel_emb, dtype=np.float32)

    d = EMB // 2
    h = ent_emb[triples[:, 0].astype(np.int64)]
    r = rel_emb[triples[:, 1].astype(np.int64)]
    re_h, im_h = h[:, :d], h[:, d:]
    re_r, im_r = r[:, :d], r[:, d:]
    ans = np.empty((B, EMB), np.float32)
    ans[:, :d] = re_h * re_r - im_h * im_r
    ans[:, d:] = re_h * im_r + im_h * re_r
    ans *= np.float32(OUT_SCALE)
    ansT_bf = np.ascontiguousarray(ans.T).astype(ml_dtypes.bfloat16)

    ent_bf = np.empty(ent_emb.shape, dtype=ml_dtypes.bfloat16)
    shards = np.empty((NCORES, EMB, SHARD), dtype=ml_dtypes.bfloat16)

    def _cast(c):
        s = slice(c * SHARD, (c + 1) * SHARD)
        ent_bf[s] = ent_emb[s]

    def _shard(c):
        shards[c] = ent_bf[c * SHARD:(c + 1) * SHARD].T

    _pmap(_cast, NCORES)
    _pmap(_shard, NCORES)
    return [{"ansT": ansT_bf, "entT": shards[c]} for c in range(NCORES)]


def run_raw(in_maps, trace=False):
    from concourse import bass_utils
    return bass_utils.run_bass_kernel_spmd(
        _get_nc(), in_maps, core_ids=list(range(NCORES)), trace=trace
    )


def assemble(results):
    out = np.empty((B, NUM_ENT), np.float32)
    inv = np.float32(1.0 / OUT_SCALE)

    def _one(c):
        sh = results[c]["score"].astype(np.float32)
        sh *= inv
        out[:, c * SHARD:(c + 1) * SHARD] = sh
    _pmap(_one, NCORES)
    return out


def kernel(triples, ent_emb, rel_emb):
    in_maps = prepare_in_maps(triples, ent_emb, rel_emb)
    res = run_raw(in_maps)
    return assemble(res.results)
